# revision 13
# baseline (speedup 1.0000x reference)
"""HRR binding self-attention kernel for 8 trn2 NeuronCores — radix-4 DFT.

Same structure as the radix-2 version, but the forward DFT is factored one
level further: x is split into 4 stride-4 subsequences whose partial DFTs
B0..B3 (512 matmul columns each, Hermitian-unique) are combined in two
slice-add levels (B0,B2 -> E block; B1,B3 -> TO block; E,TO -> packed
spectrum).  All conjugate reuse is absorbed into a permuted packing map fm,
which the host-built constants (CQ/C0/GW) are generated against.
DFT matmul cost: 8192 cy/tile (vs 16384 radix-2, 32768 direct).
"""

import sys

sys.path.insert(0, "/opt/trn_rl_repo")

import numpy as np
import ml_dtypes

import concourse.bass as bass
import concourse.bacc as bacc
import concourse.mybir as mybir
from concourse.tile import TileContext
from concourse.bass_utils import run_bass_kernel_spmd

BF16 = mybir.dt.bfloat16
F32 = mybir.dt.float32
F8 = mybir.dt.float8e4
DR = mybir.MatmulPerfMode.DoubleRow
AF = mybir.ActivationFunctionType

P = 128
D = 2048
T = 2048
NPF = 16
NT = T // P
NB = 4
NS = 4096

bf16 = ml_dtypes.bfloat16
f8 = ml_dtypes.float8_e4m3

_CACHE = {}


def _build_nc(reps: int = 1):
    nc = bacc.Bacc("TRN2", target_bir_lowering=False, debug=False, num_devices=8)
    xT = nc.dram_tensor("xT", [NT, P, NPF, P], BF16, kind="ExternalInput")
    CB = nc.dram_tensor("CB", [P, NPF, 512], BF16, kind="ExternalInput")
    GWH = nc.dram_tensor("GWH", [4, P, NPF, 512], F8, kind="ExternalInput")
    GWL = nc.dram_tensor("GWL", [4, P, NPF, 512], F8, kind="ExternalInput")
    UI = nc.dram_tensor("UI", [P, 2 * P], BF16, kind="ExternalInput")
    CQ = nc.dram_tensor("CQ", [P, NPF], F32, kind="ExternalInput")
    C0 = nc.dram_tensor("C0", [P, NPF], F32, kind="ExternalInput")
    out = nc.dram_tensor("out", [T, D], F32, kind="ExternalOutput")

    with TileContext(nc) as tc:
        with tc.tile_pool(name="misc", bufs=1) as misc:
            ui_sb = misc.tile([P, 2 * P], BF16)
            nc.sync.dma_start(ui_sb[:], UI[:])
            cq_sb = misc.tile([P, NPF], F32)
            nc.sync.dma_start(cq_sb[:], CQ[:])
            c0_sb = misc.tile([P, NPF], F32)
            nc.sync.dma_start(c0_sb[:], C0[:])

            import contextlib

            loop_ctx = tc.For_i(0, reps, 1) if reps > 1 else contextlib.nullcontext()
            with loop_ctx:
                self_body(nc, tc, ui_sb, cq_sb, c0_sb, CB, GWH, GWL, xT, out)
    nc.finalize()
    return nc


def self_body(nc, tc, ui_sb, cq_sb, c0_sb, CB, GWH, GWL, xT, out):
    with (
        tc.tile_pool(name="const", bufs=1) as cpool,
        tc.tile_pool(name="xt", bufs=4) as xpool,
        tc.tile_pool(name="xh", bufs=2) as xhpool,
        tc.tile_pool(name="eto", bufs=2) as etopool,
        tc.tile_pool(name="sbb", bufs=2) as sbbpool,
        tc.tile_pool(name="sq", bufs=3) as sqpool,
        tc.tile_pool(name="tmp", bufs=2) as tpool,
        tc.tile_pool(name="qv", bufs=2) as qvpool,
        tc.tile_pool(name="qvf", bufs=6) as qvfpool,
        tc.tile_pool(name="osb", bufs=2) as opool,
        tc.tile_pool(name="psD", bufs=2, space="PSUM") as psumD,
        tc.tile_pool(name="psT", bufs=4, space="PSUM") as psumT,
        tc.tile_pool(name="psGa", bufs=1, space="PSUM") as psumGa,
        tc.tile_pool(name="psGb", bufs=1, space="PSUM") as psumGb,
    ):
        # sync-queue order tuned so stage 1a(0) (needs xt0 + cb j=0,2) can
        # start as early as possible
        xt_hist = {}
        cb_sb = cpool.tile([P, NPF, 512], BF16)
        xt_pre = xpool.tile([P, NPF, P], BF16, tag="xt", name="xtpre0")
        nc.sync.dma_start(xt_pre[:], xT[0])
        xt_hist[0] = xt_pre
        for j in (0, 2):
            nc.sync.dma_start(
                cb_sb[:, 4 * j : 4 * j + 4, :], CB[:, 4 * j : 4 * j + 4, :]
            )
        xt_pre1 = xpool.tile([P, NPF, P], BF16, tag="xt", name="xtpre1")
        nc.sync.dma_start(xt_pre1[:], xT[1])
        xt_hist[1] = xt_pre1
        for j in (1, 3):
            nc.sync.dma_start(
                cb_sb[:, 4 * j : 4 * j + 4, :], CB[:, 4 * j : 4 * j + 4, :]
            )
        # gw hi/lo fp8 in column quarters, each its own tile (own dep
        # tracking); DMAs are emitted one pair per iteration (it=0..3)
        gwh_q = [
            cpool.tile([P, NPF, 512], F8, name=f"gwhq{q}") for q in range(4)
        ]
        gwl_q = [
            cpool.tile([P, NPF, 512], F8, name=f"gwlq{q}") for q in range(4)
        ]

        xh_hist = {}
        eto_hist = {}
        S_hist = {}
        Q_hist = {}
        qv_hist = {}

        LAG_TRI = 1
        LAG_GW = 4

        for it in range(NT + LAG_GW):
            # ---------- stage 1a: B0/B2 matmuls + E-block combine ----------
            t = it
            if t < NT:
                if t + 2 < NT:
                    xt_n = xpool.tile([P, NPF, P], BF16, tag="xt")
                    nc.sync.dma_start(xt_n[:], xT[t + 2])
                    xt_hist[t + 2] = xt_n
                xt = xt_hist.pop(t)
                xh = xhpool.tile([P, D], BF16, tag="xh")
                E_sb = etopool.tile([P, 1024], BF16, tag="E")
                psB0 = psumD.tile([P, 512], F32, tag="psD", name="psB0")
                psB2 = psumD.tile([P, 512], F32, tag="psD", name="psB2")
                for c in range(4):
                    st, sp = c == 0, c == 3
                    nc.tensor.matmul(
                        psB0[:], xt[:, c, :], cb_sb[:, c, :], start=st, stop=sp
                    )
                    nc.tensor.matmul(
                        psB2[:], xt[:, 8 + c, :], cb_sb[:, 8 + c, :], start=st, stop=sp
                    )
                sbB0 = sbbpool.tile([P, 512], F32, tag="sbB")
                nc.scalar.copy(sbB0[:], psB0[:])
                nc.vector.tensor_add(E_sb[:, 0:256], sbB0[:, 0:256], psB2[:, 0:256])
                nc.scalar.copy(E_sb[:, 256:257], sbB0[:, 256:257])
                nc.vector.tensor_sub(E_sb[:, 257:512], sbB0[:, 1:256], psB2[:, 1:256])
                nc.vector.tensor_sub(E_sb[:, 512:513], sbB0[:, 0:1], psB2[:, 0:1])
                nc.vector.tensor_add(
                    E_sb[:, 513:768], sbB0[:, 257:512], psB2[:, 256:511]
                )
                nc.scalar.copy(E_sb[:, 768:769], psB2[:, 511:512])
                nc.vector.tensor_sub(
                    E_sb[:, 769:1024], psB2[:, 256:511], sbB0[:, 257:512]
                )
                eto_hist[t] = E_sb
                xh_hist[t] = xh

            # ---------- stage 2: tri + S/Q + cmult (t - LAG_TRI) ----------
            u = it - LAG_TRI
            if 0 <= u < NT:
                xh_u = xh_hist.pop(u)
                S_sb = sqpool.tile([P, NPF, P], BF16, tag="S")
                Q_sb = sqpool.tile([P, NPF, P], BF16, tag="Q")
                for pf in range(NPF):
                    pst = psumT.tile([P, 2 * P], F32, tag="psT")
                    nc.tensor.matmul(
                        pst[:],
                        xh_u[:, pf * P : (pf + 1) * P],
                        ui_sb[:],
                        start=True,
                        stop=True,
                    )
                    carry_ap = (
                        c0_sb[:, pf : pf + 1]
                        if u == 0
                        else S_hist[u - 1][:, pf, P - 1 : P]
                    )
                    nc.scalar.activation(
                        S_sb[:, pf, :], pst[:, 0:P], AF.Identity, bias=carry_ap
                    )
                    nc.scalar.activation(
                        Q_sb[:, pf, :],
                        pst[:, P : 2 * P],
                        AF.Copy,
                        scale=cq_sb[:, pf : pf + 1],
                    )
                S_hist.pop(u - 1, None)
                S_hist[u] = S_sb
                Q_hist[u] = Q_sb

                qv = qvpool.tile([P, NPF, P], BF16, tag="qv")
                t1 = tpool.tile([P, 8, P], BF16, tag="t1")
                t2 = tpool.tile([P, 8, P], BF16, tag="t2")
                nc.vector.tensor_mul(t1[:], Q_sb[:, 0:8, :], S_sb[:, 0:8, :])
                nc.vector.tensor_mul(t2[:], Q_sb[:, 8:16, :], S_sb[:, 8:16, :])
                nc.vector.tensor_sub(qv[:, 0:8, :], t1[:], t2[:])
                t3 = tpool.tile([P, 8, P], BF16, tag="t1")
                t4 = tpool.tile([P, 8, P], BF16, tag="t2")
                nc.vector.tensor_mul(t3[:], Q_sb[:, 0:8, :], S_sb[:, 8:16, :])
                nc.vector.tensor_mul(t4[:], Q_sb[:, 8:16, :], S_sb[:, 0:8, :])
                nc.vector.tensor_add(qv[:, 8:16, :], t3[:], t4[:])
                nc.vector.tensor_mul(qv[0:1, 0, :], Q_sb[0:1, 0, :], S_sb[0:1, 0, :])
                nc.vector.tensor_mul(qv[0:1, 8, :], Q_sb[0:1, 8, :], S_sb[0:1, 8, :])
                qvh = qvfpool.tile([P, NPF, P], F8, tag="qvh")
                qvl = qvfpool.tile([P, NPF, P], F8, tag="qvl")
                nc.scalar.copy(qvh[:], qv[:])
                nc.vector.tensor_sub(qvl[:], qv[:], qvh[:])
                Q_hist.pop(u, None)
                qv_hist[u] = (qvh, qvl)

            # ---------- stage 1b: B1/B3 + TO combine + level-2 ----------
            if t < NT:
                E_sb = eto_hist.pop(t)
                TO_sb = etopool.tile([P, 1024], BF16, tag="TO")
                psB1 = psumD.tile([P, 512], F32, tag="psD", name="psB1")
                psB3 = psumD.tile([P, 512], F32, tag="psD", name="psB3")
                for c in range(4):
                    st, sp = c == 0, c == 3
                    nc.tensor.matmul(
                        psB1[:], xt[:, 4 + c, :], cb_sb[:, 4 + c, :], start=st, stop=sp
                    )
                    nc.tensor.matmul(
                        psB3[:],
                        xt[:, 12 + c, :],
                        cb_sb[:, 12 + c, :],
                        start=st,
                        stop=sp,
                    )
                sbB1 = sbbpool.tile([P, 512], F32, tag="sbB")
                nc.scalar.copy(sbB1[:], psB1[:])
                nc.vector.tensor_add(TO_sb[:, 0:256], sbB1[:, 0:256], psB3[:, 0:256])
                nc.vector.tensor_sub(
                    TO_sb[:, 256:257], psB3[:, 511:512], sbB1[:, 511:512]
                )
                nc.vector.tensor_sub(
                    TO_sb[:, 257:512], psB3[:, 256:511], sbB1[:, 256:511]
                )
                nc.vector.tensor_sub(TO_sb[:, 512:513], psB3[:, 0:1], sbB1[:, 0:1])
                nc.vector.tensor_add(
                    TO_sb[:, 513:768], sbB1[:, 256:511], psB3[:, 256:511]
                )
                nc.vector.tensor_add(
                    TO_sb[:, 768:769], sbB1[:, 511:512], psB3[:, 511:512]
                )
                nc.vector.tensor_sub(TO_sb[:, 769:1024], psB3[:, 1:256], sbB1[:, 1:256])
                # level-2 combine (both operands SBUF bf16)
                nc.vector.tensor_add(xh[:, 0:512], E_sb[:, 0:512], TO_sb[:, 0:512])
                nc.scalar.copy(xh[:, 512:513], E_sb[:, 512:513])
                nc.vector.tensor_sub(xh[:, 513:1024], E_sb[:, 1:512], TO_sb[:, 1:512])
                nc.vector.tensor_sub(xh[:, 1024:1025], E_sb[:, 0:1], TO_sb[:, 0:1])
                nc.vector.tensor_add(
                    xh[:, 1025:1536], E_sb[:, 513:1024], TO_sb[:, 513:1024]
                )
                nc.scalar.copy(xh[:, 1536:1537], TO_sb[:, 512:513])
                nc.vector.tensor_sub(
                    xh[:, 1537:2048], TO_sb[:, 513:1024], E_sb[:, 513:1024]
                )

            if it < 2:
                for q in (2 * it, 2 * it + 1):
                    nc.sync.dma_start(gwh_q[q][:], GWH[q])
                    nc.sync.dma_start(gwl_q[q][:], GWL[q])

            # ---------- stage 3: fused GW matmul (t - LAG_GW) ----------
            v = it - LAG_GW
            if v >= 0:
                qvh, qvl = qv_hist.pop(v)
                osb = opool.tile([P, D], F32, tag="osb")
                for e in range(4):
                    psg = (psumGa if e % 2 == 0 else psumGb).tile(
                        [P, 512], F32, tag="psG"
                    )
                    n_mm = 8 * 3
                    k = 0
                    for p in range(8):
                        pr = slice(2 * p, 2 * p + 2)
                        for lhs, rhs in (
                            (qvh, gwh_q[e]),
                            (qvh, gwl_q[e]),
                            (qvl, gwh_q[e]),
                        ):
                            nc.tensor.matmul(
                                psg[:],
                                lhs[:, pr, :],
                                rhs[:, pr, :],
                                start=(k == 0),
                                stop=(k == n_mm - 1),
                                perf_mode=DR,
                            )
                            k += 1
                    if e % 2 == 0:
                        nc.scalar.copy(osb[:, e * 512 : (e + 1) * 512], psg[:])
                    else:
                        nc.vector.tensor_copy(osb[:, e * 512 : (e + 1) * 512], psg[:])
                nc.sync.dma_start(out[v * P : (v + 1) * P, :], osb[:])


def _chunked(m):
    r, c = m.shape
    return np.ascontiguousarray(m.reshape(r // P, P, c).transpose(1, 0, 2))


_p = np.arange(1024)
_FM = np.where(
    _p <= 256,
    _p,
    np.where(_p <= 511, 768 - _p, np.where(_p == 512, 512,
             np.where(_p <= 768, 1536 - _p, _p - 256))),
)


def _pack_F(re, im):
    v = np.empty(2048)
    v[0:1024] = re[_FM]
    v[1024] = re[1024]
    v[1025:2048] = im[_FM[1:1024]]
    return v


def _consts():
    if "consts" in _CACHE:
        return _CACHE["consts"]

    k = np.arange(512)

    def cs_cols(dd_base, re_hi, im_lo, im_hi):
        dd = 4 * k + dd_base
        m = np.empty((512, 512))
        m[:, 0:re_hi] = np.cos(2 * np.pi * np.outer(dd, np.arange(re_hi)) / D)
        m[:, re_hi:512] = -np.sin(
            2 * np.pi * np.outer(dd, np.arange(im_lo, im_hi + 1)) / D
        )
        return m

    CB0 = cs_cols(0, 257, 1, 255)
    CB1 = cs_cols(1, 256, 1, 256)
    CB2 = cs_cols(2, 256, 1, 256)
    CB3 = cs_cols(3, 256, 1, 256)
    CBfull = np.concatenate([CB0, CB1, CB2, CB3], axis=0)  # [2048, 512]

    U = np.triu(np.ones((P, P)))
    UI = np.concatenate([U, np.eye(P)], axis=1)
    consts = {
        "CB": _chunked(CBfull.astype(np.float32)).astype(bf16),
        "UI": UI.astype(bf16),
    }
    _CACHE["consts"] = consts
    return consts


def _freq_maxqv(x, c):
    """Per-frequency bound on packed |qv| before scaling: one rfft+cumsum
    pass over the full input."""
    xf = np.fft.rfft(x.astype(np.float32), axis=-1)  # [B, S, 1025]
    kv = np.cumsum(xf, axis=1)
    qv = (xf * c) * kv
    m = np.maximum(np.abs(qv.real).max(axis=(0, 1)), np.abs(qv.imag).max(axis=(0, 1)))
    return m * 1.1 + 1e-30  # measured packed max + bf16-rounding margin


def _gw_matrix(w_out, mq):
    """Balanced per-frequency scales (qv*s and GW/s both <= ~200, fp8-e4m3
    safe), returns (GWH4, GWL4, s)."""
    f_of_row = np.empty(2048, dtype=np.int64)
    f_of_row[0:1024] = _FM
    f_of_row[1024] = 1024
    f_of_row[1025:2048] = _FM[1:1024]
    alpha = np.where((f_of_row == 0) | (f_of_row == 1024), 1.0, 2.0)
    ang = 2 * np.pi / D * np.outer(f_of_row, np.arange(D))
    G_F = np.empty((2048, D), np.float64)
    G_F[0:1025] = alpha[0:1025, None] * np.cos(ang[0:1025]) / D
    G_F[1025:] = -2.0 * np.sin(ang[1025:]) / D
    GWm = (G_F.astype(np.float32) @ w_out.T.astype(np.float32)).astype(np.float64)
    g_row = np.abs(GWm).max(axis=1) + 1e-30
    g_f = np.zeros(1025)
    np.maximum.at(g_f, f_of_row, g_row)
    s = np.sqrt(g_f / mq)
    s = np.clip(s, g_f / 200.0, 200.0 / mq)
    GWm *= 1.0 / s[f_of_row][:, None]
    GWm = np.clip(GWm, -224.0, 224.0).astype(np.float32)
    GWh = GWm.astype(f8)
    GWl = (GWm - GWh.astype(np.float32)).astype(f8)
    out = []
    for m in (GWh, GWl):
        mc = _chunked(m.astype(np.float32)).astype(f8)  # [P, NPF, D]
        m4 = np.stack([mc[:, :, q * 512 : (q + 1) * 512] for q in range(4)])
        out.append(np.ascontiguousarray(m4))
    return out[0], out[1], s


def kernel(x, queries, keyvalues, w_out):
    x = np.asarray(x, dtype=np.float32)
    queries = np.asarray(queries, dtype=np.float32)
    keyvalues = np.asarray(keyvalues, dtype=np.float32)
    w_out = np.asarray(w_out, dtype=np.float32)

    if "nc" not in _CACHE:
        _CACHE["nc"] = _build_nc()
    nc = _CACHE["nc"]
    consts = _consts()

    c = (queries * keyvalues).reshape(-1)
    mq = _freq_maxqv(x, c)
    GWHc, GWLc, s = _gw_matrix(w_out, mq)
    cs_ = c * s
    cq = _chunked(_pack_F(cs_, cs_).astype(np.float32)[:, None])[:, :, 0]

    in_maps = []
    shards = []
    for b in range(NB):
        for h in range(2):
            shards.append((b, h))
            xs = x[b, h * T : (h + 1) * T]
            xsT = np.ascontiguousarray(xs.T)
            subs = [
                _chunked(np.ascontiguousarray(xsT[j::4])) for j in range(4)
            ]  # each [P, 4, T]
            xfull = np.concatenate(subs, axis=1)  # [P, 16, T]
            xTc = np.ascontiguousarray(
                xfull.reshape(P, NPF, NT, P).transpose(2, 0, 1, 3)
            ).astype(bf16)
            if h == 0:
                c0 = np.zeros((P, NPF), np.float32)
            else:
                F = np.fft.rfft(x[b, :T].sum(axis=0).astype(np.float64))
                c0 = _chunked(
                    _pack_F(F.real, F.imag).astype(np.float32)[:, None]
                )[:, :, 0]
            in_maps.append(
                {
                    "xT": xTc,
                    "CB": consts["CB"],
                    "GWH": GWHc,
                    "GWL": GWLc,
                    "UI": consts["UI"],
                    "CQ": np.ascontiguousarray(cq),
                    "C0": np.ascontiguousarray(c0),
                }
            )

    global _LAST_IN_MAPS
    _LAST_IN_MAPS = in_maps
    res = run_bass_kernel_spmd(nc, in_maps, core_ids=list(range(8)))
    y = np.empty((NB, NS, D), np.float32)
    for i, (b, h) in enumerate(shards):
        y[b, h * T : (h + 1) * T] = res.results[i]["out"]
    return y


# revision 14
# speedup vs baseline: 1.3350x; 1.3350x over previous
"""HRR binding self-attention kernel for 8 trn2 NeuronCores — radix-4 DFT.

Same structure as the radix-2 version, but the forward DFT is factored one
level further: x is split into 4 stride-4 subsequences whose partial DFTs
B0..B3 (512 matmul columns each, Hermitian-unique) are combined in two
slice-add levels (B0,B2 -> E block; B1,B3 -> TO block; E,TO -> packed
spectrum).  All conjugate reuse is absorbed into a permuted packing map fm,
which the host-built constants (CQ/C0/GW) are generated against.
DFT matmul cost: 8192 cy/tile (vs 16384 radix-2, 32768 direct).
"""

import sys

sys.path.insert(0, "/opt/trn_rl_repo")

import numpy as np
import ml_dtypes

import concourse.bass as bass
import concourse.bacc as bacc
import concourse.mybir as mybir
from concourse.tile import TileContext
from concourse.bass_utils import run_bass_kernel_spmd

BF16 = mybir.dt.bfloat16
F32 = mybir.dt.float32
AF = mybir.ActivationFunctionType

P = 128
D = 2048
T = 2048
NPF = 16
NT = T // P
NB = 4
NS = 4096

bf16 = ml_dtypes.bfloat16

_CACHE = {}


def _build_nc(reps: int = 1):
    nc = bacc.Bacc("TRN2", target_bir_lowering=False, debug=False, num_devices=8)
    xT = nc.dram_tensor("xT", [NT, P, NPF, P], BF16, kind="ExternalInput")
    CB = nc.dram_tensor("CB", [P, NPF, 512], BF16, kind="ExternalInput")
    GW = nc.dram_tensor("GW", [4, P, NPF, 512], BF16, kind="ExternalInput")
    UI = nc.dram_tensor("UI", [P, 2 * P], BF16, kind="ExternalInput")
    CQ = nc.dram_tensor("CQ", [P, NPF], F32, kind="ExternalInput")
    C0 = nc.dram_tensor("C0", [P, NPF], F32, kind="ExternalInput")
    out = nc.dram_tensor("out", [T, D], F32, kind="ExternalOutput")

    with TileContext(nc) as tc:
        with tc.tile_pool(name="misc", bufs=1) as misc:
            ui_sb = misc.tile([P, 2 * P], BF16)
            nc.sync.dma_start(ui_sb[:], UI[:])
            cq_sb = misc.tile([P, NPF], F32)
            nc.sync.dma_start(cq_sb[:], CQ[:])
            c0_sb = misc.tile([P, NPF], F32)
            nc.sync.dma_start(c0_sb[:], C0[:])

            import contextlib

            loop_ctx = tc.For_i(0, reps, 1) if reps > 1 else contextlib.nullcontext()
            with loop_ctx:
                self_body(nc, tc, ui_sb, cq_sb, c0_sb, CB, GW, xT, out)
    nc.finalize()
    return nc


def self_body(nc, tc, ui_sb, cq_sb, c0_sb, CB, GW, xT, out):
    with (
        tc.tile_pool(name="const", bufs=1) as cpool,
        tc.tile_pool(name="xt", bufs=4) as xpool,
        tc.tile_pool(name="xh", bufs=2) as xhpool,
        tc.tile_pool(name="eto", bufs=2) as etopool,
        tc.tile_pool(name="sbb", bufs=2) as sbbpool,
        tc.tile_pool(name="sq", bufs=3) as sqpool,
        tc.tile_pool(name="tmp", bufs=2) as tpool,
        tc.tile_pool(name="qv", bufs=6) as qvpool,
        tc.tile_pool(name="osb", bufs=2) as opool,
        tc.tile_pool(name="psD", bufs=2, space="PSUM") as psumD,
        tc.tile_pool(name="psT", bufs=4, space="PSUM") as psumT,
        tc.tile_pool(name="psGa", bufs=1, space="PSUM") as psumGa,
        tc.tile_pool(name="psGb", bufs=1, space="PSUM") as psumGb,
    ):
        # sync-queue order tuned so stage 1a(0) (needs xt0 + cb j=0,2) can
        # start as early as possible
        xt_hist = {}
        cb_sb = cpool.tile([P, NPF, 512], BF16)
        xt_pre = xpool.tile([P, NPF, P], BF16, tag="xt", name="xtpre0")
        nc.sync.dma_start(xt_pre[:], xT[0])
        xt_hist[0] = xt_pre
        for j in (0, 2):
            nc.sync.dma_start(
                cb_sb[:, 4 * j : 4 * j + 4, :], CB[:, 4 * j : 4 * j + 4, :]
            )
        xt_pre1 = xpool.tile([P, NPF, P], BF16, tag="xt", name="xtpre1")
        nc.sync.dma_start(xt_pre1[:], xT[1])
        xt_hist[1] = xt_pre1
        for j in (1, 3):
            nc.sync.dma_start(
                cb_sb[:, 4 * j : 4 * j + 4, :], CB[:, 4 * j : 4 * j + 4, :]
            )
        # gw in column quarters, each its own tile (own dep tracking);
        # the DMAs are emitted one per iteration (it=0..3) inside the loop
        gw_q = [
            cpool.tile([P, NPF, 512], BF16, name=f"gwq{q}") for q in range(4)
        ]

        xh_hist = {}
        eto_hist = {}
        S_hist = {}
        Q_hist = {}
        qv_hist = {}

        LAG_TRI = 1
        LAG_GW = 4

        for it in range(NT + LAG_GW):
            # ---------- stage 1a: B0/B2 matmuls + E-block combine ----------
            t = it
            if t < NT:
                if t + 2 < NT:
                    xt_n = xpool.tile([P, NPF, P], BF16, tag="xt")
                    nc.sync.dma_start(xt_n[:], xT[t + 2])
                    xt_hist[t + 2] = xt_n
                xt = xt_hist.pop(t)
                xh = xhpool.tile([P, D], BF16, tag="xh")
                E_sb = etopool.tile([P, 1024], BF16, tag="E")
                psB0 = psumD.tile([P, 512], F32, tag="psD", name="psB0")
                psB2 = psumD.tile([P, 512], F32, tag="psD", name="psB2")
                for c in range(4):
                    st, sp = c == 0, c == 3
                    nc.tensor.matmul(
                        psB0[:], xt[:, c, :], cb_sb[:, c, :], start=st, stop=sp
                    )
                    nc.tensor.matmul(
                        psB2[:], xt[:, 8 + c, :], cb_sb[:, 8 + c, :], start=st, stop=sp
                    )
                sbB0 = sbbpool.tile([P, 512], F32, tag="sbB")
                nc.scalar.copy(sbB0[:], psB0[:])
                nc.vector.tensor_add(E_sb[:, 0:256], sbB0[:, 0:256], psB2[:, 0:256])
                nc.scalar.copy(E_sb[:, 256:257], sbB0[:, 256:257])
                nc.vector.tensor_sub(E_sb[:, 257:512], sbB0[:, 1:256], psB2[:, 1:256])
                nc.vector.tensor_sub(E_sb[:, 512:513], sbB0[:, 0:1], psB2[:, 0:1])
                nc.vector.tensor_add(
                    E_sb[:, 513:768], sbB0[:, 257:512], psB2[:, 256:511]
                )
                nc.scalar.copy(E_sb[:, 768:769], psB2[:, 511:512])
                nc.vector.tensor_sub(
                    E_sb[:, 769:1024], psB2[:, 256:511], sbB0[:, 257:512]
                )
                eto_hist[t] = E_sb
                xh_hist[t] = xh

            # ---------- stage 2: tri + S/Q + cmult (t - LAG_TRI) ----------
            u = it - LAG_TRI
            if 0 <= u < NT:
                xh_u = xh_hist.pop(u)
                S_sb = sqpool.tile([P, NPF, P], BF16, tag="S")
                Q_sb = sqpool.tile([P, NPF, P], BF16, tag="Q")
                for pf in range(NPF):
                    pst = psumT.tile([P, 2 * P], F32, tag="psT")
                    nc.tensor.matmul(
                        pst[:],
                        xh_u[:, pf * P : (pf + 1) * P],
                        ui_sb[:],
                        start=True,
                        stop=True,
                    )
                    carry_ap = (
                        c0_sb[:, pf : pf + 1]
                        if u == 0
                        else S_hist[u - 1][:, pf, P - 1 : P]
                    )
                    nc.scalar.activation(
                        S_sb[:, pf, :], pst[:, 0:P], AF.Identity, bias=carry_ap
                    )
                    nc.scalar.activation(
                        Q_sb[:, pf, :],
                        pst[:, P : 2 * P],
                        AF.Copy,
                        scale=cq_sb[:, pf : pf + 1],
                    )
                S_hist.pop(u - 1, None)
                S_hist[u] = S_sb
                Q_hist[u] = Q_sb

                qv = qvpool.tile([P, NPF, P], BF16, tag="qv")
                t1 = tpool.tile([P, 8, P], BF16, tag="t1")
                t2 = tpool.tile([P, 8, P], BF16, tag="t2")
                nc.vector.tensor_mul(t1[:], Q_sb[:, 0:8, :], S_sb[:, 0:8, :])
                nc.vector.tensor_mul(t2[:], Q_sb[:, 8:16, :], S_sb[:, 8:16, :])
                nc.vector.tensor_sub(qv[:, 0:8, :], t1[:], t2[:])
                t3 = tpool.tile([P, 8, P], BF16, tag="t1")
                t4 = tpool.tile([P, 8, P], BF16, tag="t2")
                nc.vector.tensor_mul(t3[:], Q_sb[:, 0:8, :], S_sb[:, 8:16, :])
                nc.vector.tensor_mul(t4[:], Q_sb[:, 8:16, :], S_sb[:, 0:8, :])
                nc.vector.tensor_add(qv[:, 8:16, :], t3[:], t4[:])
                nc.vector.tensor_mul(qv[0:1, 0, :], Q_sb[0:1, 0, :], S_sb[0:1, 0, :])
                nc.vector.tensor_mul(qv[0:1, 8, :], Q_sb[0:1, 8, :], S_sb[0:1, 8, :])
                Q_hist.pop(u, None)
                qv_hist[u] = qv

            # ---------- stage 1b: B1/B3 + TO combine + level-2 ----------
            if t < NT:
                E_sb = eto_hist.pop(t)
                TO_sb = etopool.tile([P, 1024], BF16, tag="TO")
                psB1 = psumD.tile([P, 512], F32, tag="psD", name="psB1")
                psB3 = psumD.tile([P, 512], F32, tag="psD", name="psB3")
                for c in range(4):
                    st, sp = c == 0, c == 3
                    nc.tensor.matmul(
                        psB1[:], xt[:, 4 + c, :], cb_sb[:, 4 + c, :], start=st, stop=sp
                    )
                    nc.tensor.matmul(
                        psB3[:],
                        xt[:, 12 + c, :],
                        cb_sb[:, 12 + c, :],
                        start=st,
                        stop=sp,
                    )
                sbB1 = sbbpool.tile([P, 512], F32, tag="sbB")
                nc.scalar.copy(sbB1[:], psB1[:])
                nc.vector.tensor_add(TO_sb[:, 0:256], sbB1[:, 0:256], psB3[:, 0:256])
                nc.vector.tensor_sub(
                    TO_sb[:, 256:257], psB3[:, 511:512], sbB1[:, 511:512]
                )
                nc.vector.tensor_sub(
                    TO_sb[:, 257:512], psB3[:, 256:511], sbB1[:, 256:511]
                )
                nc.vector.tensor_sub(TO_sb[:, 512:513], psB3[:, 0:1], sbB1[:, 0:1])
                nc.vector.tensor_add(
                    TO_sb[:, 513:768], sbB1[:, 256:511], psB3[:, 256:511]
                )
                nc.vector.tensor_add(
                    TO_sb[:, 768:769], sbB1[:, 511:512], psB3[:, 511:512]
                )
                nc.vector.tensor_sub(TO_sb[:, 769:1024], psB3[:, 1:256], sbB1[:, 1:256])
                # level-2 combine (both operands SBUF bf16)
                nc.vector.tensor_add(xh[:, 0:512], E_sb[:, 0:512], TO_sb[:, 0:512])
                nc.scalar.copy(xh[:, 512:513], E_sb[:, 512:513])
                nc.vector.tensor_sub(xh[:, 513:1024], E_sb[:, 1:512], TO_sb[:, 1:512])
                nc.vector.tensor_sub(xh[:, 1024:1025], E_sb[:, 0:1], TO_sb[:, 0:1])
                nc.vector.tensor_add(
                    xh[:, 1025:1536], E_sb[:, 513:1024], TO_sb[:, 513:1024]
                )
                nc.scalar.copy(xh[:, 1536:1537], TO_sb[:, 512:513])
                nc.vector.tensor_sub(
                    xh[:, 1537:2048], TO_sb[:, 513:1024], E_sb[:, 513:1024]
                )

            if it < 4:
                nc.sync.dma_start(gw_q[it][:], GW[it])

            # ---------- stage 3: fused GW matmul (t - LAG_GW) ----------
            v = it - LAG_GW
            if v >= 0:
                qv = qv_hist.pop(v)
                osb = opool.tile([P, D], F32, tag="osb")
                for ep in range(2):
                    psga = psumGa.tile([P, 512], F32, tag="psG", name="psga")
                    psgb = psumGb.tile([P, 512], F32, tag="psG", name="psgb")
                    for pf in range(NPF):
                        # same stationary qv chunk feeds both e-groups
                        nc.tensor.matmul(
                            psga[:],
                            qv[:, pf, :],
                            gw_q[2 * ep][:, pf, :],
                            start=(pf == 0),
                            stop=(pf == NPF - 1),
                        )
                        nc.tensor.matmul(
                            psgb[:],
                            qv[:, pf, :],
                            gw_q[2 * ep + 1][:, pf, :],
                            start=(pf == 0),
                            stop=(pf == NPF - 1),
                        )
                    nc.scalar.copy(osb[:, 2 * ep * 512 : (2 * ep + 1) * 512], psga[:])
                    nc.vector.tensor_copy(
                        osb[:, (2 * ep + 1) * 512 : (2 * ep + 2) * 512], psgb[:]
                    )
                nc.sync.dma_start(out[v * P : (v + 1) * P, :], osb[:])


def _chunked(m):
    r, c = m.shape
    return np.ascontiguousarray(m.reshape(r // P, P, c).transpose(1, 0, 2))


_p = np.arange(1024)
_FM = np.where(
    _p <= 256,
    _p,
    np.where(_p <= 511, 768 - _p, np.where(_p == 512, 512,
             np.where(_p <= 768, 1536 - _p, _p - 256))),
)


def _pack_F(re, im):
    v = np.empty(2048)
    v[0:1024] = re[_FM]
    v[1024] = re[1024]
    v[1025:2048] = im[_FM[1:1024]]
    return v


def _consts():
    if "consts" in _CACHE:
        return _CACHE["consts"]

    k = np.arange(512)

    def cs_cols(dd_base, re_hi, im_lo, im_hi):
        dd = 4 * k + dd_base
        m = np.empty((512, 512))
        m[:, 0:re_hi] = np.cos(2 * np.pi * np.outer(dd, np.arange(re_hi)) / D)
        m[:, re_hi:512] = -np.sin(
            2 * np.pi * np.outer(dd, np.arange(im_lo, im_hi + 1)) / D
        )
        return m

    CB0 = cs_cols(0, 257, 1, 255)
    CB1 = cs_cols(1, 256, 1, 256)
    CB2 = cs_cols(2, 256, 1, 256)
    CB3 = cs_cols(3, 256, 1, 256)
    CBfull = np.concatenate([CB0, CB1, CB2, CB3], axis=0)  # [2048, 512]

    U = np.triu(np.ones((P, P)))
    UI = np.concatenate([U, np.eye(P)], axis=1)
    consts = {
        "CB": _chunked(CBfull.astype(np.float32)).astype(bf16),
        "UI": UI.astype(bf16),
    }
    _CACHE["consts"] = consts
    return consts


def _gw_matrix(w_out):
    f_of_row = np.empty(2048, dtype=np.int64)
    f_of_row[0:1024] = _FM
    f_of_row[1024] = 1024
    f_of_row[1025:2048] = _FM[1:1024]
    alpha = np.where((f_of_row == 0) | (f_of_row == 1024), 1.0, 2.0)
    ang = 2 * np.pi / D * np.outer(f_of_row, np.arange(D))
    G_F = np.empty((2048, D), np.float64)
    G_F[0:1025] = alpha[0:1025, None] * np.cos(ang[0:1025]) / D
    G_F[1025:] = -2.0 * np.sin(ang[1025:]) / D
    GWm = (G_F.astype(np.float32) @ w_out.T.astype(np.float32)).astype(np.float32)
    GWc = _chunked(GWm)  # [P, NPF, D]
    GW4 = np.stack([GWc[:, :, q * 512 : (q + 1) * 512] for q in range(4)])
    return np.ascontiguousarray(GW4).astype(bf16)


def kernel(x, queries, keyvalues, w_out):
    x = np.asarray(x, dtype=np.float32)
    queries = np.asarray(queries, dtype=np.float32)
    keyvalues = np.asarray(keyvalues, dtype=np.float32)
    w_out = np.asarray(w_out, dtype=np.float32)

    if "nc" not in _CACHE:
        _CACHE["nc"] = _build_nc()
    nc = _CACHE["nc"]
    consts = _consts()

    c = (queries * keyvalues).reshape(-1)
    cq = _chunked(_pack_F(c, c).astype(np.float32)[:, None])[:, :, 0]
    GWc = _gw_matrix(w_out)

    in_maps = []
    shards = []
    for b in range(NB):
        for h in range(2):
            shards.append((b, h))
            xs = x[b, h * T : (h + 1) * T]
            xsT = np.ascontiguousarray(xs.T)
            subs = [
                _chunked(np.ascontiguousarray(xsT[j::4])) for j in range(4)
            ]  # each [P, 4, T]
            xfull = np.concatenate(subs, axis=1)  # [P, 16, T]
            xTc = np.ascontiguousarray(
                xfull.reshape(P, NPF, NT, P).transpose(2, 0, 1, 3)
            ).astype(bf16)
            if h == 0:
                c0 = np.zeros((P, NPF), np.float32)
            else:
                F = np.fft.rfft(x[b, :T].sum(axis=0).astype(np.float64))
                c0 = _chunked(
                    _pack_F(F.real, F.imag).astype(np.float32)[:, None]
                )[:, :, 0]
            in_maps.append(
                {
                    "xT": xTc,
                    "CB": consts["CB"],
                    "GW": GWc,
                    "UI": consts["UI"],
                    "CQ": np.ascontiguousarray(cq),
                    "C0": np.ascontiguousarray(c0),
                }
            )

    global _LAST_IN_MAPS
    _LAST_IN_MAPS = in_maps
    res = run_bass_kernel_spmd(nc, in_maps, core_ids=list(range(8)))
    y = np.empty((NB, NS, D), np.float32)
    for i, (b, h) in enumerate(shards):
        y[b, h * T : (h + 1) * T] = res.results[i]["out"]
    return y


# revision 15
# speedup vs baseline: 1.3828x; 1.0358x over previous
"""HRR binding self-attention kernel for 8 trn2 NeuronCores — radix-4 DFT.

Same structure as the radix-2 version, but the forward DFT is factored one
level further: x is split into 4 stride-4 subsequences whose partial DFTs
B0..B3 (512 matmul columns each, Hermitian-unique) are combined in two
slice-add levels (B0,B2 -> E block; B1,B3 -> TO block; E,TO -> packed
spectrum).  All conjugate reuse is absorbed into a permuted packing map fm,
which the host-built constants (CQ/C0/GW) are generated against.
DFT matmul cost: 8192 cy/tile (vs 16384 radix-2, 32768 direct).
"""

import sys

sys.path.insert(0, "/opt/trn_rl_repo")

import numpy as np
import ml_dtypes

import concourse.bass as bass
import concourse.bacc as bacc
import concourse.mybir as mybir
from concourse.tile import TileContext
from concourse.bass_utils import run_bass_kernel_spmd

BF16 = mybir.dt.bfloat16
F32 = mybir.dt.float32
AF = mybir.ActivationFunctionType

P = 128
D = 2048
T = 2048
NPF = 16
NT = T // P
NB = 4
NS = 4096

bf16 = ml_dtypes.bfloat16

_CACHE = {}


def _build_nc(reps: int = 1):
    nc = bacc.Bacc("TRN2", target_bir_lowering=False, debug=False, num_devices=8)
    xT = nc.dram_tensor("xT", [NT, P, NPF, P], BF16, kind="ExternalInput")
    CB = nc.dram_tensor("CB", [P, NPF, 512], BF16, kind="ExternalInput")
    GW = nc.dram_tensor("GW", [4, P, NPF, 512], BF16, kind="ExternalInput")
    UI = nc.dram_tensor("UI", [P, 2 * P], BF16, kind="ExternalInput")
    CQ = nc.dram_tensor("CQ", [P, NPF], F32, kind="ExternalInput")
    C0 = nc.dram_tensor("C0", [P, NPF], F32, kind="ExternalInput")
    out = nc.dram_tensor("out", [T, D], F32, kind="ExternalOutput")

    with TileContext(nc) as tc:
        with tc.tile_pool(name="misc", bufs=1) as misc:
            ui_sb = misc.tile([P, 2 * P], BF16)
            nc.sync.dma_start(ui_sb[:], UI[:])
            cq_sb = misc.tile([P, NPF], F32)
            nc.sync.dma_start(cq_sb[:], CQ[:])
            c0_sb = misc.tile([P, NPF], F32)
            nc.sync.dma_start(c0_sb[:], C0[:])

            import contextlib

            loop_ctx = tc.For_i(0, reps, 1) if reps > 1 else contextlib.nullcontext()
            with loop_ctx:
                self_body(nc, tc, ui_sb, cq_sb, c0_sb, CB, GW, xT, out)
    nc.finalize()
    return nc


def self_body(nc, tc, ui_sb, cq_sb, c0_sb, CB, GW, xT, out):
    with (
        tc.tile_pool(name="const", bufs=1) as cpool,
        tc.tile_pool(name="xt", bufs=4) as xpool,
        tc.tile_pool(name="xh", bufs=2) as xhpool,
        tc.tile_pool(name="eto", bufs=2) as etopool,
        tc.tile_pool(name="sbb", bufs=2) as sbbpool,
        tc.tile_pool(name="sq", bufs=3) as sqpool,
        tc.tile_pool(name="tmp", bufs=2) as tpool,
        tc.tile_pool(name="qv", bufs=6) as qvpool,
        tc.tile_pool(name="osb", bufs=2) as opool,
        tc.tile_pool(name="psD", bufs=2, space="PSUM") as psumD,
        tc.tile_pool(name="psT", bufs=4, space="PSUM") as psumT,
        tc.tile_pool(name="psGa", bufs=1, space="PSUM") as psumGa,
        tc.tile_pool(name="psGb", bufs=1, space="PSUM") as psumGb,
    ):
        # sync-queue order tuned so stage 1a(0) (needs xt0 + cb j=0,2) can
        # start as early as possible
        xt_hist = {}
        cb_sb = cpool.tile([P, NPF, 512], BF16)
        xt_pre = xpool.tile([P, NPF, P], BF16, tag="xt", name="xtpre0")
        nc.sync.dma_start(xt_pre[:], xT[0])
        xt_hist[0] = xt_pre
        for j in (0, 2):
            nc.sync.dma_start(
                cb_sb[:, 4 * j : 4 * j + 4, :], CB[:, 4 * j : 4 * j + 4, :]
            )
        xt_pre1 = xpool.tile([P, NPF, P], BF16, tag="xt", name="xtpre1")
        nc.sync.dma_start(xt_pre1[:], xT[1])
        xt_hist[1] = xt_pre1
        for j in (1, 3):
            nc.sync.dma_start(
                cb_sb[:, 4 * j : 4 * j + 4, :], CB[:, 4 * j : 4 * j + 4, :]
            )
        # gw in column quarters, each its own tile (own dep tracking);
        # the DMAs are emitted one per iteration (it=0..3) inside the loop
        gw_q = [
            cpool.tile([P, NPF, 512], BF16, name=f"gwq{q}") for q in range(4)
        ]

        xh_hist = {}
        eto_hist = {}
        S_hist = {}
        Q_hist = {}
        qv_hist = {}

        LAG_TRI = 1
        LAG_GW = 4

        for it in range(NT + LAG_GW):
            # ---------- stage 1a: B0/B2 matmuls + E-block combine ----------
            t = it
            if t < NT:
                if t + 2 < NT:
                    xt_n = xpool.tile([P, NPF, P], BF16, tag="xt")
                    nc.sync.dma_start(xt_n[:], xT[t + 2])
                    xt_hist[t + 2] = xt_n
                xt = xt_hist.pop(t)
                xh = xhpool.tile([P, D], BF16, tag="xh")
                E_sb = etopool.tile([P, 1024], BF16, tag="E")
                psB0 = psumD.tile([P, 512], F32, tag="psD", name="psB0")
                psB2 = psumD.tile([P, 512], F32, tag="psD", name="psB2")
                for c in range(4):
                    st, sp = c == 0, c == 3
                    nc.tensor.matmul(
                        psB0[:], xt[:, c, :], cb_sb[:, c, :], start=st, stop=sp
                    )
                    nc.tensor.matmul(
                        psB2[:], xt[:, 8 + c, :], cb_sb[:, 8 + c, :], start=st, stop=sp
                    )
                sbB0 = sbbpool.tile([P, 512], F32, tag="sbB")
                nc.scalar.copy(sbB0[:], psB0[:])
                nc.vector.tensor_add(E_sb[:, 0:256], sbB0[:, 0:256], psB2[:, 0:256])
                nc.scalar.copy(E_sb[:, 256:257], sbB0[:, 256:257])
                nc.vector.tensor_sub(E_sb[:, 257:512], sbB0[:, 1:256], psB2[:, 1:256])
                nc.vector.tensor_sub(E_sb[:, 512:513], sbB0[:, 0:1], psB2[:, 0:1])
                nc.vector.tensor_add(
                    E_sb[:, 513:768], sbB0[:, 257:512], psB2[:, 256:511]
                )
                nc.scalar.copy(E_sb[:, 768:769], psB2[:, 511:512])
                nc.vector.tensor_sub(
                    E_sb[:, 769:1024], psB2[:, 256:511], sbB0[:, 257:512]
                )
                eto_hist[t] = E_sb
                xh_hist[t] = xh

            # ---------- stage 2: tri + S/Q + cmult (t - LAG_TRI) ----------
            u = it - LAG_TRI
            if 0 <= u < NT:
                xh_u = xh_hist.pop(u)
                S_sb = sqpool.tile([P, NPF, P], BF16, tag="S")
                Q_sb = sqpool.tile([P, NPF, P], BF16, tag="Q")
                for pf in range(NPF):
                    pst = psumT.tile([P, 2 * P], F32, tag="psT")
                    nc.tensor.matmul(
                        pst[:],
                        xh_u[:, pf * P : (pf + 1) * P],
                        ui_sb[:],
                        start=True,
                        stop=True,
                    )
                    carry_ap = (
                        c0_sb[:, pf : pf + 1]
                        if u == 0
                        else S_hist[u - 1][:, pf, P - 1 : P]
                    )
                    nc.scalar.activation(
                        S_sb[:, pf, :], pst[:, 0:P], AF.Identity, bias=carry_ap
                    )
                    nc.vector.tensor_scalar_mul(
                        Q_sb[:, pf, :],
                        pst[:, P : 2 * P],
                        cq_sb[:, pf : pf + 1],
                    )
                S_hist.pop(u - 1, None)
                S_hist[u] = S_sb
                Q_hist[u] = Q_sb

                qv = qvpool.tile([P, NPF, P], BF16, tag="qv")
                t1 = tpool.tile([P, 8, P], BF16, tag="t1")
                t2 = tpool.tile([P, 8, P], BF16, tag="t2")
                nc.vector.tensor_mul(t1[:], Q_sb[:, 0:8, :], S_sb[:, 0:8, :])
                nc.vector.tensor_mul(t2[:], Q_sb[:, 8:16, :], S_sb[:, 8:16, :])
                nc.vector.tensor_sub(qv[:, 0:8, :], t1[:], t2[:])
                t3 = tpool.tile([P, 8, P], BF16, tag="t1")
                t4 = tpool.tile([P, 8, P], BF16, tag="t2")
                nc.vector.tensor_mul(t3[:], Q_sb[:, 0:8, :], S_sb[:, 8:16, :])
                nc.vector.tensor_mul(t4[:], Q_sb[:, 8:16, :], S_sb[:, 0:8, :])
                nc.vector.tensor_add(qv[:, 8:16, :], t3[:], t4[:])
                nc.vector.tensor_mul(qv[0:1, 0, :], Q_sb[0:1, 0, :], S_sb[0:1, 0, :])
                nc.vector.tensor_mul(qv[0:1, 8, :], Q_sb[0:1, 8, :], S_sb[0:1, 8, :])
                Q_hist.pop(u, None)
                qv_hist[u] = qv

            # ---------- stage 1b: B1/B3 + TO combine + level-2 ----------
            if t < NT:
                E_sb = eto_hist.pop(t)
                TO_sb = etopool.tile([P, 1024], BF16, tag="TO")
                psB1 = psumD.tile([P, 512], F32, tag="psD", name="psB1")
                psB3 = psumD.tile([P, 512], F32, tag="psD", name="psB3")
                for c in range(4):
                    st, sp = c == 0, c == 3
                    nc.tensor.matmul(
                        psB1[:], xt[:, 4 + c, :], cb_sb[:, 4 + c, :], start=st, stop=sp
                    )
                    nc.tensor.matmul(
                        psB3[:],
                        xt[:, 12 + c, :],
                        cb_sb[:, 12 + c, :],
                        start=st,
                        stop=sp,
                    )
                sbB1 = sbbpool.tile([P, 512], F32, tag="sbB")
                nc.scalar.copy(sbB1[:], psB1[:])
                nc.vector.tensor_add(TO_sb[:, 0:256], sbB1[:, 0:256], psB3[:, 0:256])
                nc.vector.tensor_sub(
                    TO_sb[:, 256:257], psB3[:, 511:512], sbB1[:, 511:512]
                )
                nc.vector.tensor_sub(
                    TO_sb[:, 257:512], psB3[:, 256:511], sbB1[:, 256:511]
                )
                nc.vector.tensor_sub(TO_sb[:, 512:513], psB3[:, 0:1], sbB1[:, 0:1])
                nc.vector.tensor_add(
                    TO_sb[:, 513:768], sbB1[:, 256:511], psB3[:, 256:511]
                )
                nc.vector.tensor_add(
                    TO_sb[:, 768:769], sbB1[:, 511:512], psB3[:, 511:512]
                )
                nc.vector.tensor_sub(TO_sb[:, 769:1024], psB3[:, 1:256], sbB1[:, 1:256])
                # level-2 combine (both operands SBUF bf16)
                nc.vector.tensor_add(xh[:, 0:512], E_sb[:, 0:512], TO_sb[:, 0:512])
                nc.scalar.copy(xh[:, 512:513], E_sb[:, 512:513])
                nc.vector.tensor_sub(xh[:, 513:1024], E_sb[:, 1:512], TO_sb[:, 1:512])
                nc.vector.tensor_sub(xh[:, 1024:1025], E_sb[:, 0:1], TO_sb[:, 0:1])
                nc.vector.tensor_add(
                    xh[:, 1025:1536], E_sb[:, 513:1024], TO_sb[:, 513:1024]
                )
                nc.scalar.copy(xh[:, 1536:1537], TO_sb[:, 512:513])
                nc.vector.tensor_sub(
                    xh[:, 1537:2048], TO_sb[:, 513:1024], E_sb[:, 513:1024]
                )

            if it < 4:
                nc.sync.dma_start(gw_q[it][:], GW[it])

            # ---------- stage 3: fused GW matmul (t - LAG_GW) ----------
            v = it - LAG_GW
            if v >= 0:
                qv = qv_hist.pop(v)
                osb = opool.tile([P, D], F32, tag="osb")
                for ep in range(2):
                    psga = psumGa.tile([P, 512], F32, tag="psG", name="psga")
                    psgb = psumGb.tile([P, 512], F32, tag="psG", name="psgb")
                    for pf in range(NPF):
                        # same stationary qv chunk feeds both e-groups
                        nc.tensor.matmul(
                            psga[:],
                            qv[:, pf, :],
                            gw_q[2 * ep][:, pf, :],
                            start=(pf == 0),
                            stop=(pf == NPF - 1),
                        )
                        nc.tensor.matmul(
                            psgb[:],
                            qv[:, pf, :],
                            gw_q[2 * ep + 1][:, pf, :],
                            start=(pf == 0),
                            stop=(pf == NPF - 1),
                        )
                    nc.scalar.copy(osb[:, 2 * ep * 512 : (2 * ep + 1) * 512], psga[:])
                    nc.vector.tensor_copy(
                        osb[:, (2 * ep + 1) * 512 : (2 * ep + 2) * 512], psgb[:]
                    )
                nc.sync.dma_start(out[v * P : (v + 1) * P, :], osb[:])


def _chunked(m):
    r, c = m.shape
    return np.ascontiguousarray(m.reshape(r // P, P, c).transpose(1, 0, 2))


_p = np.arange(1024)
_FM = np.where(
    _p <= 256,
    _p,
    np.where(_p <= 511, 768 - _p, np.where(_p == 512, 512,
             np.where(_p <= 768, 1536 - _p, _p - 256))),
)


def _pack_F(re, im):
    v = np.empty(2048)
    v[0:1024] = re[_FM]
    v[1024] = re[1024]
    v[1025:2048] = im[_FM[1:1024]]
    return v


def _consts():
    if "consts" in _CACHE:
        return _CACHE["consts"]

    k = np.arange(512)

    def cs_cols(dd_base, re_hi, im_lo, im_hi):
        dd = 4 * k + dd_base
        m = np.empty((512, 512))
        m[:, 0:re_hi] = np.cos(2 * np.pi * np.outer(dd, np.arange(re_hi)) / D)
        m[:, re_hi:512] = -np.sin(
            2 * np.pi * np.outer(dd, np.arange(im_lo, im_hi + 1)) / D
        )
        return m

    CB0 = cs_cols(0, 257, 1, 255)
    CB1 = cs_cols(1, 256, 1, 256)
    CB2 = cs_cols(2, 256, 1, 256)
    CB3 = cs_cols(3, 256, 1, 256)
    CBfull = np.concatenate([CB0, CB1, CB2, CB3], axis=0)  # [2048, 512]

    U = np.triu(np.ones((P, P)))
    UI = np.concatenate([U, np.eye(P)], axis=1)
    consts = {
        "CB": _chunked(CBfull.astype(np.float32)).astype(bf16),
        "UI": UI.astype(bf16),
    }
    _CACHE["consts"] = consts
    return consts


def _gw_matrix(w_out):
    f_of_row = np.empty(2048, dtype=np.int64)
    f_of_row[0:1024] = _FM
    f_of_row[1024] = 1024
    f_of_row[1025:2048] = _FM[1:1024]
    alpha = np.where((f_of_row == 0) | (f_of_row == 1024), 1.0, 2.0)
    ang = 2 * np.pi / D * np.outer(f_of_row, np.arange(D))
    G_F = np.empty((2048, D), np.float64)
    G_F[0:1025] = alpha[0:1025, None] * np.cos(ang[0:1025]) / D
    G_F[1025:] = -2.0 * np.sin(ang[1025:]) / D
    GWm = (G_F.astype(np.float32) @ w_out.T.astype(np.float32)).astype(np.float32)
    GWc = _chunked(GWm)  # [P, NPF, D]
    GW4 = np.stack([GWc[:, :, q * 512 : (q + 1) * 512] for q in range(4)])
    return np.ascontiguousarray(GW4).astype(bf16)


def kernel(x, queries, keyvalues, w_out):
    x = np.asarray(x, dtype=np.float32)
    queries = np.asarray(queries, dtype=np.float32)
    keyvalues = np.asarray(keyvalues, dtype=np.float32)
    w_out = np.asarray(w_out, dtype=np.float32)

    if "nc" not in _CACHE:
        _CACHE["nc"] = _build_nc()
    nc = _CACHE["nc"]
    consts = _consts()

    c = (queries * keyvalues).reshape(-1)
    cq = _chunked(_pack_F(c, c).astype(np.float32)[:, None])[:, :, 0]
    GWc = _gw_matrix(w_out)

    in_maps = []
    shards = []
    for b in range(NB):
        for h in range(2):
            shards.append((b, h))
            xs = x[b, h * T : (h + 1) * T]
            xsT = np.ascontiguousarray(xs.T)
            subs = [
                _chunked(np.ascontiguousarray(xsT[j::4])) for j in range(4)
            ]  # each [P, 4, T]
            xfull = np.concatenate(subs, axis=1)  # [P, 16, T]
            xTc = np.ascontiguousarray(
                xfull.reshape(P, NPF, NT, P).transpose(2, 0, 1, 3)
            ).astype(bf16)
            if h == 0:
                c0 = np.zeros((P, NPF), np.float32)
            else:
                F = np.fft.rfft(x[b, :T].sum(axis=0).astype(np.float64))
                c0 = _chunked(
                    _pack_F(F.real, F.imag).astype(np.float32)[:, None]
                )[:, :, 0]
            in_maps.append(
                {
                    "xT": xTc,
                    "CB": consts["CB"],
                    "GW": GWc,
                    "UI": consts["UI"],
                    "CQ": np.ascontiguousarray(cq),
                    "C0": np.ascontiguousarray(c0),
                }
            )

    global _LAST_IN_MAPS
    _LAST_IN_MAPS = in_maps
    res = run_bass_kernel_spmd(nc, in_maps, core_ids=list(range(8)))
    y = np.empty((NB, NS, D), np.float32)
    for i, (b, h) in enumerate(shards):
        y[b, h * T : (h + 1) * T] = res.results[i]["out"]
    return y


# revision 16
# speedup vs baseline: 1.3873x; 1.0033x over previous
"""HRR binding self-attention kernel for 8 trn2 NeuronCores — radix-4 DFT.

Same structure as the radix-2 version, but the forward DFT is factored one
level further: x is split into 4 stride-4 subsequences whose partial DFTs
B0..B3 (512 matmul columns each, Hermitian-unique) are combined in two
slice-add levels (B0,B2 -> E block; B1,B3 -> TO block; E,TO -> packed
spectrum).  All conjugate reuse is absorbed into a permuted packing map fm,
which the host-built constants (CQ/C0/GW) are generated against.
DFT matmul cost: 8192 cy/tile (vs 16384 radix-2, 32768 direct).
"""

import sys

sys.path.insert(0, "/opt/trn_rl_repo")

import numpy as np
import ml_dtypes

import concourse.bass as bass
import concourse.bacc as bacc
import concourse.mybir as mybir
from concourse.tile import TileContext
from concourse.bass_utils import run_bass_kernel_spmd

BF16 = mybir.dt.bfloat16
F32 = mybir.dt.float32
AF = mybir.ActivationFunctionType

P = 128
D = 2048
T = 2048
NPF = 16
NT = T // P
NB = 4
NS = 4096

bf16 = ml_dtypes.bfloat16

_CACHE = {}


def _build_nc(reps: int = 1):
    nc = bacc.Bacc("TRN2", target_bir_lowering=False, debug=False, num_devices=8)
    xT = nc.dram_tensor("xT", [NT, P, NPF, P], BF16, kind="ExternalInput")
    CB = nc.dram_tensor("CB", [P, NPF, 512], BF16, kind="ExternalInput")
    GW = nc.dram_tensor("GW", [4, P, NPF, 512], BF16, kind="ExternalInput")
    UI = nc.dram_tensor("UI", [P, P], BF16, kind="ExternalInput")
    C0 = nc.dram_tensor("C0", [P, NPF], F32, kind="ExternalInput")
    out = nc.dram_tensor("out", [T, D], F32, kind="ExternalOutput")

    with TileContext(nc) as tc:
        with tc.tile_pool(name="misc", bufs=1) as misc:
            ui_sb = misc.tile([P, P], BF16)
            nc.sync.dma_start(ui_sb[:], UI[:])
            c0_sb = misc.tile([P, NPF], F32)
            nc.sync.dma_start(c0_sb[:], C0[:])

            import contextlib

            loop_ctx = tc.For_i(0, reps, 1) if reps > 1 else contextlib.nullcontext()
            with loop_ctx:
                self_body(nc, tc, ui_sb, c0_sb, CB, GW, xT, out)
    nc.finalize()
    return nc


def self_body(nc, tc, ui_sb, c0_sb, CB, GW, xT, out):
    with (
        tc.tile_pool(name="const", bufs=1) as cpool,
        tc.tile_pool(name="xt", bufs=4) as xpool,
        tc.tile_pool(name="xh", bufs=2) as xhpool,
        tc.tile_pool(name="eto", bufs=2) as etopool,
        tc.tile_pool(name="sbb", bufs=2) as sbbpool,
        tc.tile_pool(name="sq", bufs=3) as sqpool,
        tc.tile_pool(name="tmp", bufs=2) as tpool,
        tc.tile_pool(name="qv", bufs=6) as qvpool,
        tc.tile_pool(name="osb", bufs=2) as opool,
        tc.tile_pool(name="psD", bufs=2, space="PSUM") as psumD,
        tc.tile_pool(name="psT", bufs=4, space="PSUM") as psumT,
        tc.tile_pool(name="psGa", bufs=1, space="PSUM") as psumGa,
        tc.tile_pool(name="psGb", bufs=1, space="PSUM") as psumGb,
    ):
        # sync-queue order tuned so stage 1a(0) (needs xt0 + cb j=0,2) can
        # start as early as possible
        xt_hist = {}
        cb_sb = cpool.tile([P, NPF, 512], BF16)
        xt_pre = xpool.tile([P, NPF, P], BF16, tag="xt", name="xtpre0")
        nc.sync.dma_start(xt_pre[:], xT[0])
        xt_hist[0] = xt_pre
        for j in (0, 2):
            nc.sync.dma_start(
                cb_sb[:, 4 * j : 4 * j + 4, :], CB[:, 4 * j : 4 * j + 4, :]
            )
        xt_pre1 = xpool.tile([P, NPF, P], BF16, tag="xt", name="xtpre1")
        nc.sync.dma_start(xt_pre1[:], xT[1])
        xt_hist[1] = xt_pre1
        for j in (1, 3):
            nc.sync.dma_start(
                cb_sb[:, 4 * j : 4 * j + 4, :], CB[:, 4 * j : 4 * j + 4, :]
            )
        # gw in column quarters, each its own tile (own dep tracking);
        # the DMAs are emitted one per iteration (it=0..3) inside the loop
        gw_q = [
            cpool.tile([P, NPF, 512], BF16, name=f"gwq{q}") for q in range(4)
        ]

        xh_hist = {}
        eto_hist = {}
        S_hist = {}
        Q_hist = {}
        qv_hist = {}

        LAG_TRI = 1
        LAG_GW = 4

        for it in range(NT + LAG_GW):
            # ---------- stage 1a: B0/B2 matmuls + E-block combine ----------
            t = it
            if t < NT:
                if t + 2 < NT:
                    xt_n = xpool.tile([P, NPF, P], BF16, tag="xt")
                    nc.sync.dma_start(xt_n[:], xT[t + 2])
                    xt_hist[t + 2] = xt_n
                xt = xt_hist.pop(t)
                xh = xhpool.tile([P, D], BF16, tag="xh")
                E_sb = etopool.tile([P, 1024], BF16, tag="E")
                psB0 = psumD.tile([P, 512], F32, tag="psD", name="psB0")
                psB2 = psumD.tile([P, 512], F32, tag="psD", name="psB2")
                for c in range(4):
                    st, sp = c == 0, c == 3
                    nc.tensor.matmul(
                        psB0[:], xt[:, c, :], cb_sb[:, c, :], start=st, stop=sp
                    )
                    nc.tensor.matmul(
                        psB2[:], xt[:, 8 + c, :], cb_sb[:, 8 + c, :], start=st, stop=sp
                    )
                sbB0 = sbbpool.tile([P, 512], F32, tag="sbB")
                nc.scalar.copy(sbB0[:], psB0[:])
                nc.vector.tensor_add(E_sb[:, 0:256], sbB0[:, 0:256], psB2[:, 0:256])
                nc.scalar.copy(E_sb[:, 256:257], sbB0[:, 256:257])
                nc.vector.tensor_sub(E_sb[:, 257:512], sbB0[:, 1:256], psB2[:, 1:256])
                nc.vector.tensor_sub(E_sb[:, 512:513], sbB0[:, 0:1], psB2[:, 0:1])
                nc.vector.tensor_add(
                    E_sb[:, 513:768], sbB0[:, 257:512], psB2[:, 256:511]
                )
                nc.scalar.copy(E_sb[:, 768:769], psB2[:, 511:512])
                nc.vector.tensor_sub(
                    E_sb[:, 769:1024], psB2[:, 256:511], sbB0[:, 257:512]
                )
                eto_hist[t] = E_sb
                xh_hist[t] = xh

            # ---------- stage 2: tri + S/Q + cmult (t - LAG_TRI) ----------
            u = it - LAG_TRI
            if 0 <= u < NT:
                xh_u = xh_hist.pop(u)
                S_sb = sqpool.tile([P, NPF, P], BF16, tag="S")
                Q_sb = sqpool.tile([P, NPF, P], BF16, tag="Q")
                # Q = spectrum transposed to freq-major via the DMA xbar; the
                # c filter is folded into the GW rows host-side
                nc.sync.dma_start_transpose(Q_sb[:], xh_u[:])
                for pf in range(NPF):
                    pst = psumT.tile([P, P], F32, tag="psT")
                    nc.tensor.matmul(
                        pst[:],
                        xh_u[:, pf * P : (pf + 1) * P],
                        ui_sb[:],
                        start=True,
                        stop=True,
                    )
                    carry_ap = (
                        c0_sb[:, pf : pf + 1]
                        if u == 0
                        else S_hist[u - 1][:, pf, P - 1 : P]
                    )
                    nc.scalar.activation(
                        S_sb[:, pf, :], pst[:], AF.Identity, bias=carry_ap
                    )
                S_hist.pop(u - 1, None)
                S_hist[u] = S_sb
                Q_hist[u] = Q_sb

                qv = qvpool.tile([P, NPF, P], BF16, tag="qv")
                t1 = tpool.tile([P, 8, P], BF16, tag="t1")
                t2 = tpool.tile([P, 8, P], BF16, tag="t2")
                nc.vector.tensor_mul(t1[:], Q_sb[:, 0:8, :], S_sb[:, 0:8, :])
                nc.vector.tensor_mul(t2[:], Q_sb[:, 8:16, :], S_sb[:, 8:16, :])
                nc.vector.tensor_sub(qv[:, 0:8, :], t1[:], t2[:])
                t3 = tpool.tile([P, 8, P], BF16, tag="t1")
                t4 = tpool.tile([P, 8, P], BF16, tag="t2")
                nc.vector.tensor_mul(t3[:], Q_sb[:, 0:8, :], S_sb[:, 8:16, :])
                nc.vector.tensor_mul(t4[:], Q_sb[:, 8:16, :], S_sb[:, 0:8, :])
                nc.vector.tensor_add(qv[:, 8:16, :], t3[:], t4[:])
                nc.vector.tensor_mul(qv[0:1, 0, :], Q_sb[0:1, 0, :], S_sb[0:1, 0, :])
                nc.vector.tensor_mul(qv[0:1, 8, :], Q_sb[0:1, 8, :], S_sb[0:1, 8, :])
                Q_hist.pop(u, None)
                qv_hist[u] = qv

            # ---------- stage 1b: B1/B3 + TO combine + level-2 ----------
            if t < NT:
                E_sb = eto_hist.pop(t)
                TO_sb = etopool.tile([P, 1024], BF16, tag="TO")
                psB1 = psumD.tile([P, 512], F32, tag="psD", name="psB1")
                psB3 = psumD.tile([P, 512], F32, tag="psD", name="psB3")
                for c in range(4):
                    st, sp = c == 0, c == 3
                    nc.tensor.matmul(
                        psB1[:], xt[:, 4 + c, :], cb_sb[:, 4 + c, :], start=st, stop=sp
                    )
                    nc.tensor.matmul(
                        psB3[:],
                        xt[:, 12 + c, :],
                        cb_sb[:, 12 + c, :],
                        start=st,
                        stop=sp,
                    )
                sbB1 = sbbpool.tile([P, 512], F32, tag="sbB")
                nc.scalar.copy(sbB1[:], psB1[:])
                nc.vector.tensor_add(TO_sb[:, 0:256], sbB1[:, 0:256], psB3[:, 0:256])
                nc.vector.tensor_sub(
                    TO_sb[:, 256:257], psB3[:, 511:512], sbB1[:, 511:512]
                )
                nc.vector.tensor_sub(
                    TO_sb[:, 257:512], psB3[:, 256:511], sbB1[:, 256:511]
                )
                nc.vector.tensor_sub(TO_sb[:, 512:513], psB3[:, 0:1], sbB1[:, 0:1])
                nc.vector.tensor_add(
                    TO_sb[:, 513:768], sbB1[:, 256:511], psB3[:, 256:511]
                )
                nc.vector.tensor_add(
                    TO_sb[:, 768:769], sbB1[:, 511:512], psB3[:, 511:512]
                )
                nc.vector.tensor_sub(TO_sb[:, 769:1024], psB3[:, 1:256], sbB1[:, 1:256])
                # level-2 combine (both operands SBUF bf16)
                nc.vector.tensor_add(xh[:, 0:512], E_sb[:, 0:512], TO_sb[:, 0:512])
                nc.scalar.copy(xh[:, 512:513], E_sb[:, 512:513])
                nc.vector.tensor_sub(xh[:, 513:1024], E_sb[:, 1:512], TO_sb[:, 1:512])
                nc.vector.tensor_sub(xh[:, 1024:1025], E_sb[:, 0:1], TO_sb[:, 0:1])
                nc.vector.tensor_add(
                    xh[:, 1025:1536], E_sb[:, 513:1024], TO_sb[:, 513:1024]
                )
                nc.scalar.copy(xh[:, 1536:1537], TO_sb[:, 512:513])
                nc.vector.tensor_sub(
                    xh[:, 1537:2048], TO_sb[:, 513:1024], E_sb[:, 513:1024]
                )

            if it < 4:
                nc.sync.dma_start(gw_q[it][:], GW[it])

            # ---------- stage 3: fused GW matmul (t - LAG_GW) ----------
            v = it - LAG_GW
            if v >= 0:
                qv = qv_hist.pop(v)
                osb = opool.tile([P, D], F32, tag="osb")
                for ep in range(2):
                    psga = psumGa.tile([P, 512], F32, tag="psG", name="psga")
                    psgb = psumGb.tile([P, 512], F32, tag="psG", name="psgb")
                    for pf in range(NPF):
                        # same stationary qv chunk feeds both e-groups
                        nc.tensor.matmul(
                            psga[:],
                            qv[:, pf, :],
                            gw_q[2 * ep][:, pf, :],
                            start=(pf == 0),
                            stop=(pf == NPF - 1),
                        )
                        nc.tensor.matmul(
                            psgb[:],
                            qv[:, pf, :],
                            gw_q[2 * ep + 1][:, pf, :],
                            start=(pf == 0),
                            stop=(pf == NPF - 1),
                        )
                    nc.scalar.copy(osb[:, 2 * ep * 512 : (2 * ep + 1) * 512], psga[:])
                    nc.vector.tensor_copy(
                        osb[:, (2 * ep + 1) * 512 : (2 * ep + 2) * 512], psgb[:]
                    )
                nc.sync.dma_start(out[v * P : (v + 1) * P, :], osb[:])


def _chunked(m):
    r, c = m.shape
    return np.ascontiguousarray(m.reshape(r // P, P, c).transpose(1, 0, 2))


_p = np.arange(1024)
_FM = np.where(
    _p <= 256,
    _p,
    np.where(_p <= 511, 768 - _p, np.where(_p == 512, 512,
             np.where(_p <= 768, 1536 - _p, _p - 256))),
)


def _pack_F(re, im):
    v = np.empty(2048)
    v[0:1024] = re[_FM]
    v[1024] = re[1024]
    v[1025:2048] = im[_FM[1:1024]]
    return v


def _consts():
    if "consts" in _CACHE:
        return _CACHE["consts"]

    k = np.arange(512)

    def cs_cols(dd_base, re_hi, im_lo, im_hi):
        dd = 4 * k + dd_base
        m = np.empty((512, 512))
        m[:, 0:re_hi] = np.cos(2 * np.pi * np.outer(dd, np.arange(re_hi)) / D)
        m[:, re_hi:512] = -np.sin(
            2 * np.pi * np.outer(dd, np.arange(im_lo, im_hi + 1)) / D
        )
        return m

    CB0 = cs_cols(0, 257, 1, 255)
    CB1 = cs_cols(1, 256, 1, 256)
    CB2 = cs_cols(2, 256, 1, 256)
    CB3 = cs_cols(3, 256, 1, 256)
    CBfull = np.concatenate([CB0, CB1, CB2, CB3], axis=0)  # [2048, 512]

    U = np.triu(np.ones((P, P)))
    consts = {
        "CB": _chunked(CBfull.astype(np.float32)).astype(bf16),
        "UI": U.astype(bf16),
    }
    _CACHE["consts"] = consts
    return consts


def _gw_matrix(w_out, c):
    f_of_row = np.empty(2048, dtype=np.int64)
    f_of_row[0:1024] = _FM
    f_of_row[1024] = 1024
    f_of_row[1025:2048] = _FM[1:1024]
    alpha = np.where((f_of_row == 0) | (f_of_row == 1024), 1.0, 2.0)
    ang = 2 * np.pi / D * np.outer(f_of_row, np.arange(D))
    G_F = np.empty((2048, D), np.float64)
    G_F[0:1025] = alpha[0:1025, None] * np.cos(ang[0:1025]) / D
    G_F[1025:] = -2.0 * np.sin(ang[1025:]) / D
    G_F *= c[f_of_row][:, None]  # spectral filter folded into the irfft rows
    GWm = (G_F.astype(np.float32) @ w_out.T.astype(np.float32)).astype(np.float32)
    GWc = _chunked(GWm)  # [P, NPF, D]
    GW4 = np.stack([GWc[:, :, q * 512 : (q + 1) * 512] for q in range(4)])
    return np.ascontiguousarray(GW4).astype(bf16)


def kernel(x, queries, keyvalues, w_out):
    x = np.asarray(x, dtype=np.float32)
    queries = np.asarray(queries, dtype=np.float32)
    keyvalues = np.asarray(keyvalues, dtype=np.float32)
    w_out = np.asarray(w_out, dtype=np.float32)

    if "nc" not in _CACHE:
        _CACHE["nc"] = _build_nc()
    nc = _CACHE["nc"]
    consts = _consts()

    c = (queries * keyvalues).reshape(-1)
    GWc = _gw_matrix(w_out, c)

    in_maps = []
    shards = []
    for b in range(NB):
        for h in range(2):
            shards.append((b, h))
            xs = x[b, h * T : (h + 1) * T]
            xsT = np.ascontiguousarray(xs.T)
            subs = [
                _chunked(np.ascontiguousarray(xsT[j::4])) for j in range(4)
            ]  # each [P, 4, T]
            xfull = np.concatenate(subs, axis=1)  # [P, 16, T]
            xTc = np.ascontiguousarray(
                xfull.reshape(P, NPF, NT, P).transpose(2, 0, 1, 3)
            ).astype(bf16)
            if h == 0:
                c0 = np.zeros((P, NPF), np.float32)
            else:
                F = np.fft.rfft(x[b, :T].sum(axis=0).astype(np.float64))
                c0 = _chunked(
                    _pack_F(F.real, F.imag).astype(np.float32)[:, None]
                )[:, :, 0]
            in_maps.append(
                {
                    "xT": xTc,
                    "CB": consts["CB"],
                    "GW": GWc,
                    "UI": consts["UI"],
                    "C0": np.ascontiguousarray(c0),
                }
            )

    global _LAST_IN_MAPS
    _LAST_IN_MAPS = in_maps
    res = run_bass_kernel_spmd(nc, in_maps, core_ids=list(range(8)))
    y = np.empty((NB, NS, D), np.float32)
    for i, (b, h) in enumerate(shards):
        y[b, h * T : (h + 1) * T] = res.results[i]["out"]
    return y
